# revision 8
# baseline (speedup 1.0000x reference)
"""DeepSeek-V3.1 MoE block (B=2,S=512,H=1024,I=512,E=64,topK=8) on 8 trn2 cores.

Strategy (expert-parallel, sparse dispatch, bf16 streaming):
  - The reference's dense-masked MoE is mathematically top-8 sparse: only the
    top-8 experts per token contribute (mask is 0 elsewhere). We exploit that.
  - Host: router (fp64 numpy, selection margin on this regime is ~4e-6 >>
    rounding noise), top-8 per token, per-expert token gather with capacity
    padding (C = multiple of 32, >= max per-expert load).
  - Device, per core c: 8 experts (count-sorted assignment). All matmul data
    (X, weights) is cast to bf16 on host — halves the HBM weight traffic that
    dominates this kernel (~25 MB/core) and runs the PE at full bf16 rate.
    Per expert slot the three weight matrices are packed into ONE dram tensor
    [128, 3*4096] and fetched as two ~1.5 MB DMAs, one on each HWDGE ring
    (sync + scalar), so both rings stream weights continuously.
    Y_e = (silu(X_e @ Wg) * (X_e @ Wu) * w_route) @ Wd accumulated in fp32
    PSUM. Shared expert: token-parallel (each core takes 128 of 1024 tokens).
  - Host: scatter-add per-expert outputs back by token, add shared.
"""
import os as _os, sys
try:
    import concourse  # noqa: F401  (env-provided, e.g. axon boot path)
except ImportError:
    for _p in ('/root/.axon_site/_ro/trn_rl_repo', '/opt/trn_rl_repo'):
        if _os.path.isdir(_p) and _p not in sys.path:
            sys.path.append(_p)
import numpy as np
import ml_dtypes

BF16NP = ml_dtypes.bfloat16

B, S, H, I, E, TOPK = 2, 512, 1024, 512, 64, 8
T = B * S
NCORES = 8
ELOC = E // NCORES
HC, IC = H // 128, I // 128
HCI = HC * I  # columns per packed weight matrix (4096)
TSH = T // NCORES  # shared-expert tokens per core (128)

LAST_RESULT = None  # BassKernelResults of the most recent run (for test harness)


def _pmajor(a, nchunk):
    """[nchunk*128, F] -> partition-major [128, nchunk*F] (chunk-row-major)."""
    F = a.shape[1]
    return np.ascontiguousarray(
        a.reshape(nchunk, 128, F).transpose(1, 0, 2).reshape(128, nchunk * F))


def _build(caps, repeat=1):
    import contextlib
    import concourse.bacc as bacc
    import concourse.mybir as mybir
    from concourse import tile, masks

    F32 = mybir.dt.float32
    BF16 = mybir.dt.bfloat16
    SILU = mybir.ActivationFunctionType.Silu

    # per-slot capacities (counts-sorted assignment): slot el holds capacity
    # caps[el]; flat tensors are concatenations over slots.
    xoff = np.concatenate([[0], np.cumsum([HC * c for c in caps])])
    yoff = np.concatenate([[0], np.cumsum(caps)])
    nb = [(c + 127) // 128 for c in caps]
    boff = np.concatenate([[0], np.cumsum(nb)])
    XW, YW, NBT = int(xoff[-1]), int(yoff[-1]), int(boff[-1])

    nc = bacc.Bacc("TRN2", target_bir_lowering=False, debug=False)

    xg_d = nc.dram_tensor("xg", [128, XW], BF16, kind="ExternalInput")
    w_d = nc.dram_tensor("w", [ELOC, 128, 3 * HCI], BF16, kind="ExternalInput")
    cf_d = nc.dram_tensor("cf", [128, NBT], F32, kind="ExternalInput")
    xs_d = nc.dram_tensor("xs", [128, HC * TSH], BF16, kind="ExternalInput")
    ws_d = nc.dram_tensor("ws", [128, 3 * HCI], BF16, kind="ExternalInput")
    yg_d = nc.dram_tensor("yg", [YW, H], BF16, kind="ExternalOutput")
    ys_d = nc.dram_tensor("ys", [TSH, H], BF16, kind="ExternalOutput")

    with tile.TileContext(nc) as tc:
        with (
            tc.tile_pool(name="const", bufs=1) as cpool,
            tc.tile_pool(name="wp", bufs=4) as wpool,
            tc.tile_pool(name="xp", bufs=4) as xpool,
            tc.tile_pool(name="ap", bufs=3) as apool,
            tc.tile_pool(name="ps", bufs=2, space="PSUM") as pspool,
        ):
            ident = cpool.tile([128, 128], BF16)
            masks.make_identity(nc, ident[:])

            cf_all = cpool.tile([128, NBT], F32)
            nc.gpsimd.dma_start(cf_all[:], cf_d[:])

            def ffn_block(xg_t, w_t, rows, r0, C_in, coef_ap, out_ap):
                """One <=128-row block through SwiGLU + down-proj.

                xg_t: [128, HC*C_in] X^T bf16; w_t [128, 3*HCI] packed
                wg|wu|wd partition-major bf16; coef_ap [rows,1] routing weight
                per row (or None); out_ap DRAM [rows,H] fp32.
                """
                g_ps = pspool.tile([128, I], F32, tag="g")
                u_ps = pspool.tile([128, I], F32, tag="u")
                for h in range(HC):
                    nc.tensor.matmul(g_ps[:rows], xg_t[:, h * C_in + r0:h * C_in + r0 + rows],
                                     w_t[:, h * I:(h + 1) * I],
                                     start=(h == 0), stop=(h == HC - 1))
                for h in range(HC):
                    nc.tensor.matmul(u_ps[:rows], xg_t[:, h * C_in + r0:h * C_in + r0 + rows],
                                     w_t[:, HCI + h * I:HCI + (h + 1) * I],
                                     start=(h == 0), stop=(h == HC - 1))
                s_sb = apool.tile([128, I], F32, tag="s")
                nc.scalar.activation(s_sb[:rows], g_ps[:rows], SILU)
                a_bf = apool.tile([128, I], BF16, tag="a")
                if coef_ap is not None:
                    nc.vector.tensor_mul(s_sb[:rows], s_sb[:rows], u_ps[:rows])
                    nc.vector.tensor_scalar_mul(a_bf[:rows], s_sb[:rows], coef_ap)
                else:
                    nc.vector.tensor_mul(a_bf[:rows], s_sb[:rows], u_ps[:rows])
                at_sb = apool.tile([128, IC * 128], BF16, tag="at")
                for i in range(IC):
                    t_ps = pspool.tile([128, 128], BF16, tag="t")
                    nc.tensor.transpose(t_ps[:, :rows],
                                        a_bf[:rows, i * 128:(i + 1) * 128],
                                        ident[:rows, :rows])
                    nc.vector.tensor_copy(at_sb[:, i * 128:i * 128 + rows], t_ps[:, :rows])
                y_sb = apool.tile([128, H], BF16, tag="ysb")
                for half in range(2):
                    y_ps = pspool.tile([128, 512], F32, tag="y")
                    for i in range(IC):
                        nc.tensor.matmul(y_ps[:rows], at_sb[:, i * 128:i * 128 + rows],
                                         w_t[:, 2 * HCI + i * H + 512 * half:
                                             2 * HCI + i * H + 512 * (half + 1)],
                                         start=(i == 0), stop=(i == IC - 1))
                    nc.vector.tensor_copy(y_sb[:rows, 512 * half:512 * (half + 1)], y_ps[:rows])
                nc.gpsimd.dma_start(out_ap, y_sb[:rows])

            HALF = 3 * HCI // 2  # 6144: ring split point (wg + wu/2 | wu/2 + wd)
            # measurement-only: in-NEFF repetition of the whole body so a
            # single (expensive) axon execute amortizes R device spans.
            rep_ctx = (tc.For_i(0, repeat, 1) if repeat > 1
                       else contextlib.nullcontext())
            with rep_ctx:
              for e in range(ELOC):
                C = caps[e]
                blocks = [(r0, min(128, C - r0)) for r0 in range(0, C, 128)]
                w_t = wpool.tile([128, 3 * HCI], BF16, tag="w")
                xg_t = xpool.tile([128, HC * max(caps)], BF16, tag="xg")
                nc.gpsimd.dma_start(xg_t[:, :HC * C], xg_d[:, xoff[e]:xoff[e + 1]])
                nc.sync.dma_start(w_t[:, :HALF], w_d[e][:, :HALF])
                nc.scalar.dma_start(w_t[:, HALF:], w_d[e][:, HALF:])
                for b, (r0, rows) in enumerate(blocks):
                    ffn_block(xg_t, w_t, rows, r0, C,
                              cf_all[:rows, boff[e] + b:boff[e] + b + 1],
                              yg_d[yoff[e] + r0:yoff[e] + r0 + rows, :])

              # shared expert on this core's token slice
              ws_t = wpool.tile([128, 3 * HCI], BF16, tag="w")
              xs_t = xpool.tile([128, HC * TSH], BF16, tag="xg")
              nc.gpsimd.dma_start(xs_t[:], xs_d[:])
              nc.sync.dma_start(ws_t[:, :HALF], ws_d[:, :HALF])
              nc.scalar.dma_start(ws_t[:, HALF:], ws_d[:, HALF:])
              ffn_block(xs_t, ws_t, TSH, 0, TSH, None, ys_d[:, :])

    nc.compile()
    return nc


def prepare(hidden_states, router_w, shared_gate_w, shared_up_w, shared_down_w,
            expert_gate_k, expert_up_k, expert_down_k):
    """Host-side routing + dispatch. Returns (nc, in_maps, meta)."""
    x = np.ascontiguousarray(np.asarray(hidden_states, dtype=np.float32).reshape(T, H))
    rw = np.asarray(router_w, dtype=np.float32)
    egk = np.asarray(expert_gate_k, dtype=np.float32)
    euk = np.asarray(expert_up_k, dtype=np.float32)
    edk = np.asarray(expert_down_k, dtype=np.float32)
    sgw = np.asarray(shared_gate_w, dtype=np.float32)
    suw = np.asarray(shared_up_w, dtype=np.float32)
    sdw = np.asarray(shared_down_w, dtype=np.float32)

    # ---- routing on host (fp64; selection margin >> fp32 noise) ----
    logits = x.astype(np.float64) @ rw.astype(np.float64)
    aff = 1.0 / (1.0 + np.exp(-logits))
    top_idx = np.argpartition(-aff, TOPK - 1, axis=1)[:, :TOPK]        # [T,8]
    top_vals = np.take_along_axis(aff, top_idx, axis=1)
    top_w = top_vals / (top_vals.sum(axis=1, keepdims=True) + 1e-9)    # [T,8]

    flat_e = top_idx.ravel()
    flat_t = np.repeat(np.arange(T), TOPK)
    flat_w = top_w.ravel()
    order = np.argsort(flat_e, kind="stable")
    se, st, sw = flat_e[order], flat_t[order], flat_w[order]
    counts = np.bincount(flat_e, minlength=E)
    offs = np.concatenate([[0], np.cumsum(counts)])

    # count-sorted assignment: slot el gets the el-th group of 8 heaviest
    # experts (one per core) -> light slots get smaller capacities.
    perm = np.argsort(-counts, kind="stable")          # experts by load desc
    slot_expert = perm.reshape(ELOC, NCORES)           # [slot, core] -> expert
    caps = [int(max(32, -(-counts[slot_expert[el]].max() // 32) * 32))
            for el in range(ELOC)]
    nb = [(c + 127) // 128 for c in caps]
    xoff = np.concatenate([[0], np.cumsum([HC * c for c in caps])])
    yoff = np.concatenate([[0], np.cumsum(caps)])
    boff = np.concatenate([[0], np.cumsum(nb)])

    nc = _build(caps, repeat=int(_os.environ.get("KERNEL_REPEAT", "1")))

    xbf = x.astype(BF16NP)
    ws_all = np.concatenate(
        [_pmajor(sgw, HC), _pmajor(suw, HC), _pmajor(sdw, IC)], axis=1
    ).astype(BF16NP)

    in_maps = []
    for c in range(NCORES):
        xg = np.zeros((128, int(xoff[-1])), BF16NP)
        cf = np.zeros((128, int(boff[-1])), np.float32)
        for el in range(ELOC):
            e = int(slot_expert[el, c])
            C = caps[el]
            toks = st[offs[e]:offs[e + 1]]
            ws = sw[offs[e]:offs[e + 1]]
            n = len(toks)
            xe = np.zeros((C, H), BF16NP)
            xe[:n] = xbf[toks]
            xg[:, xoff[el]:xoff[el + 1]] = _pmajor(np.ascontiguousarray(xe.T), HC)
            cfp = np.zeros(nb[el] * 128, np.float32)
            cfp[:n] = ws
            cf[:, boff[el]:boff[el + 1]] = cfp.reshape(nb[el], 128).T

        def wstack(w, nchunk):  # [ELOC, nchunk*128, F] -> [ELOC, 128, nchunk*F]
            F = w.shape[2]
            return np.ascontiguousarray(
                w.reshape(ELOC, nchunk, 128, F).transpose(0, 2, 1, 3)
                 .reshape(ELOC, 128, nchunk * F))

        eids = slot_expert[:, c]
        w_all = np.concatenate(
            [wstack(np.ascontiguousarray(egk[eids]), HC),
             wstack(np.ascontiguousarray(euk[eids]), HC),
             wstack(np.ascontiguousarray(edk[eids]), IC)], axis=2
        ).astype(BF16NP)
        in_maps.append({
            "xg": xg,
            "w": w_all,
            "cf": cf,
            "xs": _pmajor(np.ascontiguousarray(x[TSH * c:TSH * (c + 1)].T), HC
                          ).astype(BF16NP),
            "ws": ws_all,
        })

    return nc, in_maps, (st, offs, slot_expert, yoff)


def assemble(results, meta):
    st, offs, slot_expert, yoff = meta
    out = np.zeros((T, H), np.float32)
    for c in range(NCORES):
        r = results[c]
        out[TSH * c:TSH * (c + 1)] += r["ys"].astype(np.float32)
        yg = r["yg"]
        for el in range(ELOC):
            e = int(slot_expert[el, c])
            toks = st[offs[e]:offs[e + 1]]
            out[toks] += yg[yoff[el]:yoff[el] + len(toks)].astype(np.float32)
    return out.reshape(B, S, H)


def kernel(**inputs):
    global LAST_RESULT
    import os, time
    from concourse.bass_utils import run_bass_kernel_spmd
    if os.environ.get("BASS_TRACE"):
        try:
            import antenv.axon_hooks  # noqa: F401
        except ImportError:
            # trace requested but the axon NTFF hook module isn't present in
            # this container -- tracing would crash mid-run; disable it.
            os.environ["BASS_NEVER_TRACE"] = "1"
    nc, in_maps, meta = prepare(**inputs)
    last_err = None
    for attempt in range(3):
        try:
            res = run_bass_kernel_spmd(nc, in_maps, core_ids=list(range(NCORES)))
            break
        except Exception as err:  # transient device faults (e.g. NRT exec errors)
            last_err = err
            time.sleep(5 * (attempt + 1))
    else:
        raise last_err
    LAST_RESULT = res
    return assemble(res.results, meta)


# revision 12
# speedup vs baseline: 202.3460x; 202.3460x over previous
"""DeepSeek-V3.1 MoE block (B=2,S=512,H=1024,I=512,E=64,topK=8) on 8 trn2 cores.

Strategy (expert-parallel, sparse dispatch, bf16 streaming):
  - The reference's dense-masked MoE is mathematically top-8 sparse: only the
    top-8 experts per token contribute (mask is 0 elsewhere). We exploit that.
  - Host: router (fp64 numpy, selection margin on this regime is ~4e-6 >>
    rounding noise), top-8 per token, per-expert token gather with capacity
    padding (C = multiple of 32, >= max per-expert load).
  - Device, per core c: 8 experts (count-sorted assignment). All matmul data
    (X, weights) is cast to bf16 on host — halves the HBM weight traffic that
    dominates this kernel (~25 MB/core) and runs the PE at full bf16 rate.
    Per expert slot the three weight matrices are packed into ONE dram tensor
    [128, 3*4096] and fetched as two ~1.5 MB DMAs, one on each HWDGE ring
    (sync + scalar), so both rings stream weights continuously.
    Y_e = (silu(X_e @ Wg) * (X_e @ Wu) * w_route) @ Wd accumulated in fp32
    PSUM. Shared expert: token-parallel (each core takes 128 of 1024 tokens).
  - Host: scatter-add per-expert outputs back by token, add shared.
"""
import os as _os, sys
try:
    import concourse  # noqa: F401  (env-provided, e.g. axon boot path)
except ImportError:
    for _p in ('/root/.axon_site/_ro/trn_rl_repo', '/opt/trn_rl_repo'):
        if _os.path.isdir(_p) and _p not in sys.path:
            sys.path.append(_p)
import numpy as np
import ml_dtypes

BF16NP = ml_dtypes.bfloat16

B, S, H, I, E, TOPK = 2, 512, 1024, 512, 64, 8
T = B * S
NCORES = 8
ELOC = E // NCORES
HC, IC = H // 128, I // 128
HCI = HC * I  # columns per packed weight matrix (4096)
TSH = T // NCORES  # shared-expert tokens per core (128)

LAST_RESULT = None  # BassKernelResults of the most recent run (for test harness)


def _pmajor(a, nchunk):
    """[nchunk*128, F] -> partition-major [128, nchunk*F] (chunk-row-major)."""
    F = a.shape[1]
    return np.ascontiguousarray(
        a.reshape(nchunk, 128, F).transpose(1, 0, 2).reshape(128, nchunk * F))


def _build(caps, repeat=1):
    import contextlib
    import concourse.bacc as bacc
    import concourse.mybir as mybir
    from concourse import tile, masks

    F32 = mybir.dt.float32
    BF16 = mybir.dt.bfloat16
    SILU = mybir.ActivationFunctionType.Silu

    # per-slot capacities (counts-sorted assignment): slot el holds capacity
    # caps[el]; flat tensors are concatenations over slots.
    xoff = np.concatenate([[0], np.cumsum([HC * c for c in caps])])
    yoff = np.concatenate([[0], np.cumsum(caps)])
    nb = [(c + 127) // 128 for c in caps]
    boff = np.concatenate([[0], np.cumsum(nb)])
    XW, YW, NBT = int(xoff[-1]), int(yoff[-1]), int(boff[-1])

    nc = bacc.Bacc("TRN2", target_bir_lowering=False, debug=False)

    xg_d = nc.dram_tensor("xg", [128, XW], BF16, kind="ExternalInput")
    w_d = nc.dram_tensor("w", [ELOC, 128, 3 * HCI], BF16, kind="ExternalInput")
    cf_d = nc.dram_tensor("cf", [128, NBT], F32, kind="ExternalInput")
    xs_d = nc.dram_tensor("xs", [128, HC * TSH], BF16, kind="ExternalInput")
    ws_d = nc.dram_tensor("ws", [128, 3 * HCI], BF16, kind="ExternalInput")
    yg_d = nc.dram_tensor("yg", [YW, H], BF16, kind="ExternalOutput")
    ys_d = nc.dram_tensor("ys", [TSH, H], BF16, kind="ExternalOutput")

    with tile.TileContext(nc) as tc:
        with (
            tc.tile_pool(name="const", bufs=1) as cpool,
            tc.tile_pool(name="wp", bufs=4) as wpool,
            tc.tile_pool(name="xp", bufs=4) as xpool,
            tc.tile_pool(name="ap", bufs=3) as apool,
            tc.tile_pool(name="ps", bufs=2, space="PSUM") as pspool,
        ):
            ident = cpool.tile([128, 128], BF16)
            masks.make_identity(nc, ident[:])

            cf_all = cpool.tile([128, NBT], F32)
            nc.gpsimd.dma_start(cf_all[:], cf_d[:])

            def ffn_block(xg_t, w_t, rows, r0, C_in, coef_ap, out_ap):
                """One <=128-row block through SwiGLU + down-proj.

                xg_t: [128, HC*C_in] X^T bf16; w_t [128, 3*HCI] packed
                wg|wu|wd partition-major bf16; coef_ap [rows,1] routing weight
                per row (or None); out_ap DRAM [rows,H] fp32.
                """
                g_ps = pspool.tile([128, I], F32, tag="g")
                u_ps = pspool.tile([128, I], F32, tag="u")
                for h in range(HC):
                    nc.tensor.matmul(g_ps[:rows], xg_t[:, h * C_in + r0:h * C_in + r0 + rows],
                                     w_t[:, h * I:(h + 1) * I],
                                     start=(h == 0), stop=(h == HC - 1))
                for h in range(HC):
                    nc.tensor.matmul(u_ps[:rows], xg_t[:, h * C_in + r0:h * C_in + r0 + rows],
                                     w_t[:, HCI + h * I:HCI + (h + 1) * I],
                                     start=(h == 0), stop=(h == HC - 1))
                s_sb = apool.tile([128, I], F32, tag="s")
                nc.scalar.activation(s_sb[:rows], g_ps[:rows], SILU)
                a_bf = apool.tile([128, I], BF16, tag="a")
                if coef_ap is not None:
                    nc.vector.tensor_mul(s_sb[:rows], s_sb[:rows], u_ps[:rows])
                    nc.vector.tensor_scalar_mul(a_bf[:rows], s_sb[:rows], coef_ap)
                else:
                    nc.vector.tensor_mul(a_bf[:rows], s_sb[:rows], u_ps[:rows])
                at_sb = apool.tile([128, IC * 128], BF16, tag="at")
                for i in range(IC):
                    t_ps = pspool.tile([128, 128], BF16, tag="t")
                    nc.tensor.transpose(t_ps[:, :rows],
                                        a_bf[:rows, i * 128:(i + 1) * 128],
                                        ident[:rows, :rows])
                    nc.vector.tensor_copy(at_sb[:, i * 128:i * 128 + rows], t_ps[:, :rows])
                y_sb = apool.tile([128, H], BF16, tag="ysb")
                for half in range(2):
                    y_ps = pspool.tile([128, 512], F32, tag="y")
                    for i in range(IC):
                        nc.tensor.matmul(y_ps[:rows], at_sb[:, i * 128:i * 128 + rows],
                                         w_t[:, 2 * HCI + i * H + 512 * half:
                                             2 * HCI + i * H + 512 * (half + 1)],
                                         start=(i == 0), stop=(i == IC - 1))
                    nc.vector.tensor_copy(y_sb[:rows, 512 * half:512 * (half + 1)], y_ps[:rows])
                nc.gpsimd.dma_start(out_ap, y_sb[:rows])

            HALF = 3 * HCI // 2  # 6144: ring split point (wg + wu/2 | wu/2 + wd)
            # measurement-only: in-NEFF repetition of the whole body so a
            # single (expensive) axon execute amortizes R device spans.
            rep_ctx = (tc.For_i(0, repeat, 1) if repeat > 1
                       else contextlib.nullcontext())
            with rep_ctx:
              for e in range(ELOC):
                C = caps[e]
                blocks = [(r0, min(128, C - r0)) for r0 in range(0, C, 128)]
                w_t = wpool.tile([128, 3 * HCI], BF16, tag="w")
                xg_t = xpool.tile([128, HC * max(caps)], BF16, tag="xg")
                nc.gpsimd.dma_start(xg_t[:, :HC * C], xg_d[:, xoff[e]:xoff[e + 1]])
                nc.sync.dma_start(w_t[:, :HCI], w_d[e][:, :HCI])
                nc.sync.dma_start(w_t[:, HCI:HALF], w_d[e][:, HCI:HALF])
                nc.scalar.dma_start(w_t[:, HALF:2 * HCI], w_d[e][:, HALF:2 * HCI])
                nc.scalar.dma_start(w_t[:, 2 * HCI:], w_d[e][:, 2 * HCI:])
                for b, (r0, rows) in enumerate(blocks):
                    ffn_block(xg_t, w_t, rows, r0, C,
                              cf_all[:rows, boff[e] + b:boff[e] + b + 1],
                              yg_d[yoff[e] + r0:yoff[e] + r0 + rows, :])

              # shared expert on this core's token slice
              ws_t = wpool.tile([128, 3 * HCI], BF16, tag="w")
              xs_t = xpool.tile([128, HC * TSH], BF16, tag="xg")
              nc.gpsimd.dma_start(xs_t[:], xs_d[:])
              nc.sync.dma_start(ws_t[:, :HCI], ws_d[:, :HCI])
              nc.sync.dma_start(ws_t[:, HCI:HALF], ws_d[:, HCI:HALF])
              nc.scalar.dma_start(ws_t[:, HALF:2 * HCI], ws_d[:, HALF:2 * HCI])
              nc.scalar.dma_start(ws_t[:, 2 * HCI:], ws_d[:, 2 * HCI:])
              ffn_block(xs_t, ws_t, TSH, 0, TSH, None, ys_d[:, :])

    nc.compile()
    return nc


def prepare(hidden_states, router_w, shared_gate_w, shared_up_w, shared_down_w,
            expert_gate_k, expert_up_k, expert_down_k):
    """Host-side routing + dispatch. Returns (nc, in_maps, meta)."""
    x = np.ascontiguousarray(np.asarray(hidden_states, dtype=np.float32).reshape(T, H))
    rw = np.asarray(router_w, dtype=np.float32)
    egk = np.asarray(expert_gate_k, dtype=np.float32)
    euk = np.asarray(expert_up_k, dtype=np.float32)
    edk = np.asarray(expert_down_k, dtype=np.float32)
    sgw = np.asarray(shared_gate_w, dtype=np.float32)
    suw = np.asarray(shared_up_w, dtype=np.float32)
    sdw = np.asarray(shared_down_w, dtype=np.float32)

    # ---- routing on host (fp64; selection margin >> fp32 noise) ----
    logits = x.astype(np.float64) @ rw.astype(np.float64)
    aff = 1.0 / (1.0 + np.exp(-logits))
    top_idx = np.argpartition(-aff, TOPK - 1, axis=1)[:, :TOPK]        # [T,8]
    top_vals = np.take_along_axis(aff, top_idx, axis=1)
    top_w = top_vals / (top_vals.sum(axis=1, keepdims=True) + 1e-9)    # [T,8]

    flat_e = top_idx.ravel()
    flat_t = np.repeat(np.arange(T), TOPK)
    flat_w = top_w.ravel()
    order = np.argsort(flat_e, kind="stable")
    se, st, sw = flat_e[order], flat_t[order], flat_w[order]
    counts = np.bincount(flat_e, minlength=E)
    offs = np.concatenate([[0], np.cumsum(counts)])

    # count-sorted assignment: slot el gets the el-th group of 8 heaviest
    # experts (one per core) -> light slots get smaller capacities.
    perm = np.argsort(-counts, kind="stable")          # experts by load desc
    slot_expert = perm.reshape(ELOC, NCORES)           # [slot, core] -> expert
    caps = [int(max(32, -(-counts[slot_expert[el]].max() // 32) * 32))
            for el in range(ELOC)]
    nb = [(c + 127) // 128 for c in caps]
    xoff = np.concatenate([[0], np.cumsum([HC * c for c in caps])])
    yoff = np.concatenate([[0], np.cumsum(caps)])
    boff = np.concatenate([[0], np.cumsum(nb)])

    nc = _build(caps, repeat=int(_os.environ.get("KERNEL_REPEAT", "1")))

    xbf = x.astype(BF16NP)
    ws_all = np.concatenate(
        [_pmajor(sgw, HC), _pmajor(suw, HC), _pmajor(sdw, IC)], axis=1
    ).astype(BF16NP)

    in_maps = []
    for c in range(NCORES):
        xg = np.zeros((128, int(xoff[-1])), BF16NP)
        cf = np.zeros((128, int(boff[-1])), np.float32)
        for el in range(ELOC):
            e = int(slot_expert[el, c])
            C = caps[el]
            toks = st[offs[e]:offs[e + 1]]
            ws = sw[offs[e]:offs[e + 1]]
            n = len(toks)
            xe = np.zeros((C, H), BF16NP)
            xe[:n] = xbf[toks]
            xg[:, xoff[el]:xoff[el + 1]] = _pmajor(np.ascontiguousarray(xe.T), HC)
            cfp = np.zeros(nb[el] * 128, np.float32)
            cfp[:n] = ws
            cf[:, boff[el]:boff[el + 1]] = cfp.reshape(nb[el], 128).T

        def wstack(w, nchunk):  # [ELOC, nchunk*128, F] -> [ELOC, 128, nchunk*F]
            F = w.shape[2]
            return np.ascontiguousarray(
                w.reshape(ELOC, nchunk, 128, F).transpose(0, 2, 1, 3)
                 .reshape(ELOC, 128, nchunk * F))

        eids = slot_expert[:, c]
        w_all = np.concatenate(
            [wstack(np.ascontiguousarray(egk[eids]), HC),
             wstack(np.ascontiguousarray(euk[eids]), HC),
             wstack(np.ascontiguousarray(edk[eids]), IC)], axis=2
        ).astype(BF16NP)
        in_maps.append({
            "xg": xg,
            "w": w_all,
            "cf": cf,
            "xs": _pmajor(np.ascontiguousarray(x[TSH * c:TSH * (c + 1)].T), HC
                          ).astype(BF16NP),
            "ws": ws_all,
        })

    return nc, in_maps, (st, offs, slot_expert, yoff)


def assemble(results, meta):
    st, offs, slot_expert, yoff = meta
    out = np.zeros((T, H), np.float32)
    for c in range(NCORES):
        r = results[c]
        out[TSH * c:TSH * (c + 1)] += r["ys"].astype(np.float32)
        yg = r["yg"]
        for el in range(ELOC):
            e = int(slot_expert[el, c])
            toks = st[offs[e]:offs[e + 1]]
            out[toks] += yg[yoff[el]:yoff[el] + len(toks)].astype(np.float32)
    return out.reshape(B, S, H)


def kernel(**inputs):
    global LAST_RESULT
    import os, time
    from concourse.bass_utils import run_bass_kernel_spmd
    if os.environ.get("BASS_TRACE"):
        try:
            import antenv.axon_hooks  # noqa: F401
        except ImportError:
            # trace requested but the axon NTFF hook module isn't present in
            # this container -- tracing would crash mid-run; disable it.
            os.environ["BASS_NEVER_TRACE"] = "1"
    nc, in_maps, meta = prepare(**inputs)
    last_err = None
    for attempt in range(3):
        try:
            res = run_bass_kernel_spmd(nc, in_maps, core_ids=list(range(NCORES)))
            break
        except Exception as err:  # transient device faults (e.g. NRT exec errors)
            last_err = err
            time.sleep(5 * (attempt + 1))
    else:
        raise last_err
    LAST_RESULT = res
    return assemble(res.results, meta)
